# revision 1
# baseline (speedup 1.0000x reference)
"""Trainium2 Bass kernel for nn_AbilityGammaAttention.

Reference computation (per batch b):
    ws = s_j @ Ws_w.T + Ws_b                      # (P, A)
    uh = exp_tokens @ U_w.T                       # (Q, LE, A)
    e[q,p,t] = v . tanh(uh[q,t,:] + ws[p,:])      # (Q, P, LE)
    e masked by exp_mask (tokens), joint softmax over (Q, LE) per (b, p)
    out[q,p,:] = sum_t a[q,p,t] * exp_tokens[q,t,:], zeroed where req_mask[p]==0

Sharding: data-parallel over B across the 8 NeuronCores (batch b -> core b).

Per-core design:
  - uh is computed transposed (uhT: [A=128 partitions, tokens]) so the per-p
    "+ ws[p,:]" is a per-partition bias fused into the ScalarE tanh (the
    ScalarE tanh over P x tokens x A elements is the bottleneck engine).
  - e for all 32 p is accumulated directly into a PSUM tile [P, tokens-half]
    using a selector weight matrix (column p of slice p = v, rest zeros), so
    per-p PSUM evacuation is avoided; exp_mask lands as an extra accumulated
    (m-1)*1e9 rank-1 matmul, exactly reproducing the reference -1e9 masking.
  - Tokens are processed in two halves with the x-load/transpose/uh prep of
    half 1 overlapping the tanh loop of half 0.
  - Matmuls run in float32r (TF32-like); the tanh->e path runs in bf16
    (tanh output is in [-1,1]).
  - Softmax uses a data-independent shift (bound = sum|v_w|, computed on the
    host at first call) instead of a row max, so exp can never overflow and
    both reduce_max passes disappear; denominators come free via the ScalarE
    Exp accum_out, and the normalization (and req_mask) is folded into the
    per-partition scale of the PSUM->SBUF output copies.
  - exp_mask is exploited by HOST-side per-q token compaction: each q's
    unmasked tokens are packed to the front and padded to `le` slots, where
    le is chosen ADAPTIVELY at call time as the actual max unmasked count
    rounded up to a multiple of 8 (clamped to [64, 128]). Masked/padded
    slots contribute exactly 0, so this is mathematically exact, and the
    e-accumulation tolerates a partial final PSUM bank chunk. For ~50%%
    random masks this cuts the ScalarE tanh roofline by ~37%%.
"""

import sys

if "/opt/trn_rl_repo" not in sys.path:
    sys.path.insert(0, "/opt/trn_rl_repo")

import numpy as np

import concourse.bacc as bacc
import concourse.mybir as mybir
from concourse.masks import make_identity
from concourse.tile import TileContext

F32 = mybir.dt.float32
F32R = mybir.dt.float32r
BF16 = mybir.dt.bfloat16
I32 = mybir.dt.int32
AF = mybir.ActivationFunctionType
ALU = mybir.AluOpType

B, Q, LE, D, P, A = 8, 32, 128, 512, 32, 128
N_CORES = 8


def build_kernel(q=Q, bound=12.0, le=LE):
    """Build the per-core kernel (one batch per core). q must be a multiple of 8.

    `bound` is any value >= max possible |e| = sum(|v_w|); exp is shifted by it
    instead of a computed row max (softmax is shift-invariant, and exp(e-bound)
    can never overflow)."""
    T = q * le          # tokens per batch
    GW = 4 * le         # uh-group width (4 q per group)
    DC = D // 128       # contraction chunks (4)
    T2 = T // 2         # tokens per half
    QH = q // 2         # q per half
    assert le % 8 == 0 and 64 <= le <= 128 and QH % 4 == 0
    # per-half e chunks: full PSUM banks plus a partial final bank (each bank
    # holds exactly one accumulation group either way)
    chunks = [(o, min(512, T2 - o)) for o in range(0, T2, 512)]
    xbufs = 3 if len(chunks) <= 3 else 2   # PSUM budget: xtp + ups + e-banks <= 8

    nc = bacc.Bacc("TRN2", target_bir_lowering=False, debug=False)

    x_dram = nc.dram_tensor("exp_tokens", [q, le, D], F32, kind="ExternalInput")
    mr_dram = nc.dram_tensor("m_row_in", [1, T], F32, kind="ExternalInput")
    sj_dram = nc.dram_tensor("s_j", [P, D], F32, kind="ExternalInput")
    rm_dram = nc.dram_tensor("req_mask", [P], I32, kind="ExternalInput")
    wsw_dram = nc.dram_tensor("Ws_w", [A, D], F32, kind="ExternalInput")
    wsb_dram = nc.dram_tensor("Ws_b", [A], F32, kind="ExternalInput")
    uw_dram = nc.dram_tensor("U_w", [A, D], F32, kind="ExternalInput")
    vw_dram = nc.dram_tensor("v_w", [1, A], F32, kind="ExternalInput")
    out_dram = nc.dram_tensor("out", [q, P, D], F32, kind="ExternalOutput")

    with TileContext(nc) as tc:
        with tc.tile_pool(name="live", bufs=1) as L:
            # ---- whole-kernel tensors (base of the SBUF stack) ----------
            ident = L.tile([128, 128], F32)
            identr = L.tile([128, 128], F32R)
            x_all = L.tile([128, q * D], F32R)     # x[i] natural at cols i*D
            uhT = L.tile([A, T], F32)              # U_w @ x.T
            wsT = L.tile([A, P], F32)
            vsel_b = L.tile([A, P * P], BF16)
            m_row = L.tile([1, T], F32R)           # (m-1)*1e9
            ones_r = L.tile([1, P], F32R)
            rm_f = L.tile([P, 1], F32)
            e_full = L.tile([P, T], F32)
            aT_all = L.tile([128, Q * P], F32R)
            nsc = 6 if le <= 96 else 4
            scs = [L.tile([A, T // 2], BF16, name=f"sc{j}") for j in range(nsc)]
            sumh = [L.tile([P, 1], F32, name=f"sumh{j}") for j in range(2)]
            nbnd = L.tile([P, 1], F32)
            sums = L.tile([P, 1], F32)
            rc = L.tile([P, 1], F32)
            rc2 = L.tile([P, 1], F32)

            make_identity(nc, ident)
            nc.vector.tensor_copy(identr[:], ident[:])
            nc.gpsimd.memset(nbnd[:], -float(bound))

            with (
                tc.tile_pool(name="prep", bufs=1) as C,
                tc.tile_pool(name="ps0", bufs=1, space="PSUM") as P0,
                tc.tile_pool(name="pse", bufs=1, space="PSUM") as PE_,
            ):
                # ---- params ---------------------------------------------
                uw_sb = C.tile([A, D], F32)
                wsw_sb = C.tile([A, D], F32)
                sj_sb = C.tile([P, D], F32)
                wsb_sb = C.tile([A, 1], F32)
                v_sb = C.tile([A, 1], F32)
                nc.sync.dma_start(uw_sb[:], uw_dram[:])
                nc.sync.dma_start(wsw_sb[:], wsw_dram[:])
                nc.sync.dma_start(sj_sb[:], sj_dram[:])
                nc.sync.dma_start(wsb_sb[:, 0:1], wsb_dram.ap().rearrange("(a o) -> a o", o=1))
                nc.sync.dma_start(v_sb[:, 0:1], vw_dram.ap().rearrange("o a -> a o"))

                uwT_r = C.tile([128, DC * A], F32R)
                wswT = C.tile([128, DC * A], F32)
                sjT = C.tile([128, DC * P], F32)
                for c in range(DC):
                    tp = P0.tile([128, 128], F32, tag="xtp", bufs=xbufs)
                    nc.tensor.transpose(tp[:], uw_sb[:, c * 128:(c + 1) * 128], ident[:])
                    nc.vector.tensor_copy(uwT_r[:, c * A:(c + 1) * A], tp[:])
                    tp2 = P0.tile([128, 128], F32, tag="xtp", bufs=xbufs)
                    nc.tensor.transpose(tp2[:], wsw_sb[:, c * 128:(c + 1) * 128], ident[:])
                    nc.vector.tensor_copy(wswT[:, c * A:(c + 1) * A], tp2[:])
                    tp3 = P0.tile([128, P], F32, tag="xtp", bufs=xbufs)
                    nc.tensor.transpose(tp3[:], sj_sb[:, c * 128:(c + 1) * 128], ident[0:P, 0:P])
                    nc.vector.tensor_copy(sjT[:, c * P:(c + 1) * P], tp3[:])

                # ws.T = Ws_w @ s_j.T + Ws_b : [A partitions, P]
                ws_ps = P0.tile([A, P], F32, tag="ups", bufs=2)
                for c in range(DC):
                    nc.tensor.matmul(
                        ws_ps[:], wswT[:, c * A:(c + 1) * A], sjT[:, c * P:(c + 1) * P],
                        start=(c == 0), stop=(c == DC - 1),
                    )
                nc.vector.tensor_scalar_add(wsT[:], ws_ps[:], wsb_sb[:, 0:1])

                # selector weights: slice p has column p = v
                vsel_f = C.tile([A, P * P], F32)
                nc.gpsimd.memset(vsel_f[:], 0.0)
                for p in range(P):
                    nc.vector.tensor_copy(vsel_f[:, p * P + p:p * P + p + 1], v_sb[:, 0:1])
                nc.vector.tensor_copy(vsel_b[:], vsel_f[:])

                # mask row (m-1)*1e9 is precomputed on the host
                nc.sync.dma_start(m_row[:], mr_dram.ap().bitcast(F32R))
                ones_f = C.tile([1, P], F32)
                nc.gpsimd.memset(ones_f[:], 1.0)
                nc.vector.tensor_copy(ones_r[:], ones_f[:])
                rm_i = C.tile([P, 1], I32)
                nc.sync.dma_start(rm_i[:, 0:1], rm_dram.ap().rearrange("(p o) -> p o", o=1))
                nc.vector.tensor_copy(rm_f[:, 0:1], rm_i[:, 0:1])

                # ---- per-half: load x, build uhT, tanh+e loop -----------
                with tc.tile_pool(name="xts", bufs=1) as XT:
                    for h in range(2):
                        q0 = h * QH
                        for i in range(q0, q0 + QH):
                            nc.sync.dma_start(
                                x_all[0:le, i * D:(i + 1) * D], x_dram[i].bitcast(F32R))
                        for g in range(QH // 4):
                            xt_c = []
                            for c in range(DC):
                                tp = P0.tile([128, GW], F32, tag="xtp", bufs=xbufs)
                                for j in range(4):
                                    iq = q0 + g * 4 + j
                                    nc.tensor.transpose(
                                        tp[:, j * le:(j + 1) * le].bitcast(F32R),
                                        x_all[0:le, iq * D + c * 128: iq * D + (c + 1) * 128],
                                        identr[0:le, 0:le],
                                    )
                                xs = XT.tile([128, GW], F32R, tag=f"xs{c}")
                                nc.vector.tensor_copy(xs[:], tp[:])
                                xt_c.append(xs)
                            ups = P0.tile([A, GW], F32, tag="ups", bufs=2)
                            for c in range(DC):
                                nc.tensor.matmul(
                                    ups[:], uwT_r[:, c * A:(c + 1) * A], xt_c[c][:],
                                    start=(c == 0), stop=(c == DC - 1),
                                )
                            gi = (h * (QH // 4) + g)
                            nc.vector.tensor_copy(uhT[:, gi * GW:(gi + 1) * GW], ups[:])

                        # tanh + selector-matmul accumulation for this half
                        T2h = T // 2
                        e_ps = PE_.tile([P, T2h], F32, tag="eps", bufs=1)
                        for p in range(P):
                            sc = scs[p % nsc]
                            nc.scalar.activation(
                                sc[:], uhT[:, h * T2h:(h + 1) * T2h], AF.Tanh,
                                bias=wsT[:, p:p + 1], scale=1.0,
                            )
                            for off, w in chunks:
                                nc.tensor.matmul(
                                    e_ps[:, off:off + w],
                                    vsel_b[:, p * P:(p + 1) * P],
                                    sc[:, off:off + w],
                                    start=(p == 0), stop=False,
                                )
                        # additive exp_mask: e += 1 * ((m-1)*1e9)
                        for off, w in chunks:
                            nc.tensor.matmul(
                                e_ps[:, off:off + w],
                                ones_r[:, 0:P],
                                m_row[:, h * T2h + off: h * T2h + off + w],
                                start=False, stop=True,
                            )
                        # exp(e - bound) evacuates PSUM and accumulates the
                        # half-denominator in one ScalarE pass
                        nc.scalar.activation(
                            e_full[:, h * T2h:(h + 1) * T2h], e_ps[:], AF.Exp,
                            bias=nbnd[:, 0:1], scale=1.0,
                            accum_out=sumh[h][:, 0:1],
                        )

            # ---- softmax normalization scalar (applied on output copies) ---
            nc.vector.tensor_tensor(sums[:, 0:1], sumh[0][:, 0:1], sumh[1][:, 0:1], op=ALU.add)
            nc.vector.reciprocal(rc[:, 0:1], sums[:, 0:1])
            nc.vector.tensor_tensor(rc2[:, 0:1], rc[:, 0:1], rm_f[:, 0:1], op=ALU.mult)

            # ---- apply: out[i] = a[:, i-block] @ x[i] -------------------
            with (
                tc.tile_pool(name="apl", bufs=3) as AP_,
                tc.tile_pool(name="psa", bufs=2, space="PSUM") as PA,
            ):
                for i in range(q):
                    atp = PA.tile([128, P], F32, tag="atp", bufs=4)
                    nc.tensor.transpose(
                        atp[0:le, :], e_full[:, i * le:(i + 1) * le], ident[0:P, 0:P])
                    nc.vector.tensor_copy(aT_all[0:le, i * P:(i + 1) * P], atp[0:le, :])
                    ops = PA.tile([P, D], F32, tag="ops", bufs=4)
                    nc.tensor.matmul(ops[:], aT_all[0:le, i * P:(i + 1) * P],
                                     x_all[0:le, i * D:(i + 1) * D],
                                     start=True, stop=True)
                    osb = AP_.tile([P, D], F32, tag="osb", bufs=6)
                    if i % 3 == 0:
                        nc.vector.tensor_scalar_mul(osb[:], ops[:], rc2[:, 0:1])
                    else:
                        nc.scalar.activation(osb[:], ops[:], AF.Copy,
                                             bias=0.0, scale=rc2[:, 0:1])
                    nc.sync.dma_start(out_dram[i], osb[:])

    nc.compile()
    return nc


_NC_CACHE = {}
LAST_NC = None


def _get_nc(q=Q, bound=12.0, le=LE):
    key = (q, round(float(bound), 6), le)
    if key not in _NC_CACHE:
        _NC_CACHE[key] = build_kernel(q, bound, le)
    return _NC_CACHE[key]


def _compact(exp_tokens, exp_mask, le):
    """Per-(b,q) host compaction: move each q's unmasked tokens to the front,
    pad to `le` slots (padding slots masked out). Exact: masked tokens never
    contribute to softmax or output. Returns None if any q overflows `le`."""
    b, q, full, d = exp_tokens.shape
    counts = exp_mask.sum(axis=2)
    if counts.max() > le:
        return None
    x_c = np.empty((b, q, le, d), dtype=np.float32)
    m_c = np.zeros((b, q, le), dtype=np.float32)
    for bi in range(b):
        for qi in range(q):
            idx = np.flatnonzero(exp_mask[bi, qi])
            n = len(idx)
            x_c[bi, qi, :n] = exp_tokens[bi, qi, idx]
            if n < le:
                x_c[bi, qi, n:] = 0.0
            m_c[bi, qi, :n] = 1.0
    m_row = ((m_c.reshape(b, 1, q * le) - 1.0) * 1e9).astype(np.float32)
    return x_c, m_row


def kernel(exp_tokens, exp_mask, s_j, req_mask, Ws_w, Ws_b, U_w, v_w):
    """Full-input entry point: shard over B across 8 cores, gather output."""
    from concourse.bass_utils import run_bass_kernel_spmd

    exp_tokens = np.asarray(exp_tokens, dtype=np.float32)
    exp_mask = np.asarray(exp_mask, dtype=np.int32)
    bound = float(np.abs(np.asarray(v_w, dtype=np.float64)).sum()) + 1.0

    le = int(min(128, max(64, -(-int(exp_mask.sum(axis=2).max()) // 8) * 8)))
    packed = _compact(exp_tokens, exp_mask, le)
    if packed is None:
        # improbable overflow (>96 of 128 tokens unmasked somewhere):
        # fall back to the uncompacted kernel
        le = LE
        m_c = ((exp_mask.reshape(B, 1, Q * LE).astype(np.float32) - 1.0) * 1e9)
        packed = (exp_tokens, m_c)
    x_c, m_row = packed

    nc = _get_nc(Q, bound, le)
    global LAST_NC
    LAST_NC = nc
    in_maps = []
    for b in range(N_CORES):
        in_maps.append({
            "exp_tokens": np.ascontiguousarray(x_c[b], dtype=np.float32),
            "m_row_in": np.ascontiguousarray(m_row[b], dtype=np.float32),
            "s_j": np.ascontiguousarray(s_j[b], dtype=np.float32),
            "req_mask": np.ascontiguousarray(req_mask[b], dtype=np.int32),
            "Ws_w": np.ascontiguousarray(Ws_w, dtype=np.float32),
            "Ws_b": np.ascontiguousarray(Ws_b, dtype=np.float32),
            "U_w": np.ascontiguousarray(U_w, dtype=np.float32),
            "v_w": np.ascontiguousarray(v_w, dtype=np.float32),
        })
    res = run_bass_kernel_spmd(nc, in_maps, core_ids=list(range(N_CORES)))
    out = np.stack([res.results[b]["out"] for b in range(N_CORES)], axis=0)
    return out.astype(np.float32)


def reference_1b(x, m, sj, rm, Ws_w, Ws_b, U_w, v_w):
    """Numpy reference for ONE batch, mirroring the kernel's math (fp64)."""
    q = x.shape[0]
    T = q * LE
    xf = x.reshape(T, D).astype(np.float64)
    ws = sj.astype(np.float64) @ Ws_w.T.astype(np.float64) + Ws_b.astype(np.float64)
    uh = xf @ U_w.T.astype(np.float64)                       # [T, A]
    mf = m.reshape(T).astype(np.float64)
    e = np.tanh(uh[None, :, :] + ws[:, None, :]) @ v_w[0].astype(np.float64)  # [P, T]
    em = e + (mf[None, :] - 1.0) * 1e9
    bnd = np.abs(v_w).sum() + 1.0
    exm = np.exp(em - bnd)
    a = exm / exm.sum(axis=1, keepdims=True) * rm.astype(np.float64)[:, None]
    out = np.zeros((q, P, D))
    for i in range(q):
        out[i] = a[:, i * LE:(i + 1) * LE] @ xf[i * LE:(i + 1) * LE]
    return out



# revision 13
# speedup vs baseline: 1.7380x; 1.7380x over previous
"""Trainium2 Bass kernel for nn_AbilityGammaAttention.

Reference computation (per batch b):
    ws = s_j @ Ws_w.T + Ws_b                      # (P, A)
    uh = exp_tokens @ U_w.T                       # (Q, LE, A)
    e[q,p,t] = v . tanh(uh[q,t,:] + ws[p,:])      # (Q, P, LE)
    e masked by exp_mask (tokens), joint softmax over (Q, LE) per (b, p)
    out[q,p,:] = sum_t a[q,p,t] * exp_tokens[q,t,:], zeroed where req_mask[p]==0

Sharding: data-parallel over B across the 8 NeuronCores (batch b -> core b).

Key idea (replaces the per-p tanh loop of the previous version): expand the
shifted-tanh family in a fixed basis,

    tanh(u + w) ~= c0(w) + clin(w)*u + sum_r c_r(w) * tanh(u + b_r),

with R=12 fixed shifts b_r. The c*(w) coefficient functions are least-squares
fits (precomputed on a w-grid at import; Gaussian-weighted in u with a uniform
floor for tail control). Since ws = s_j@Ws_w.T + Ws_b is host-computable, the
host evaluates the coefficients at the actual w values and uploads, per core,
stationary matrices S_r[a,p] = v_a * c_r(ws[p,a]). On device:

    e[p,t] = sum_r (S_r^T @ tanh(uhT + b_r))[p,t] + (S_lin^T @ uhT)[p,t] + mask

i.e. R ScalarE tanh passes over [A, T] + (R+2) PE matmuls, instead of P=32
tanh passes. The c0 term is a per-p constant -> cancels in the softmax over t
(the denominators come from the Exp pass's accum_out and the normalization +
req_mask is applied on the host).

Other structure:
  - Host-side token compaction with a per-rank profile: each batch's queries
    are sorted by unmasked-token count; slot i is padded to the max count at
    rank i across batches (shared static shape, ~18% fewer tokens than
    uniform padding). Padding slots are masked via an additive (m-1)*1e9
    rank-1 matmul, exactly like the reference -1e9 masking.
  - All heavy dataflow in bf16: x is uploaded bf16 and XBAR-transpose-DMA'd
    into xT (no on-device transposes for uh), uh/tanh/coefficients/apply all
    bf16 (f32 PSUM accumulation).
  - Output: per-slot apply matmuls pack 3 slots per PSUM bank (base
    partitions 0/32/64), evacuated bf16 and DMA'd out unnormalized; the host
    divides by the denominators and applies req_mask.
"""

import sys

if "/opt/trn_rl_repo" not in sys.path:
    sys.path.insert(0, "/opt/trn_rl_repo")

import numpy as np
import ml_dtypes

import concourse.bacc as bacc
import concourse.mybir as mybir
from concourse.masks import make_identity
from concourse.tile import TileContext

F32 = mybir.dt.float32
F32R = mybir.dt.float32r
BF16 = mybir.dt.bfloat16
AF = mybir.ActivationFunctionType

B, Q, LE, D, P, A = 8, 32, 128, 512, 32, 128
N_CORES = 8
DC = D // 128
R = 12                       # tanh basis size
DEBUG_DUMP = False
STAGE = 3
import os as _os
KSKIP = set(_os.environ.get('KSKIP','').split(','))  # debug: 0=uh only, 1=+tanh, 2=+e/exp, 3=full
SEG_CAP = 1536               # max tokens per segment (3 PSUM banks of f32)

# ---------------------------------------------------------------------------
# basis fit (data-independent; computed once at import)
# ---------------------------------------------------------------------------


def _build_fit(r=R, ridge=1e-6):
    b_r = np.linspace(-4.4, 4.4, r)
    u = np.linspace(-6.5, 6.5, 1301)
    rho = np.exp(-0.5 * u * u) + 0.01
    rho /= rho.sum()
    Phi = np.concatenate(
        [np.ones_like(u)[:, None], u[:, None], np.tanh(u[:, None] + b_r[None, :])],
        axis=1)
    M = Phi.T @ (rho[:, None] * Phi) + ridge * np.eye(r + 2)
    w_grid = np.linspace(-5.0, 5.0, 2001)
    G = np.tanh(u[:, None] + w_grid[None, :])
    C_grid = np.linalg.solve(M, Phi.T @ (rho[:, None] * G))   # [(r+2), Nw]
    return b_r, w_grid, C_grid


_B_R, _W_GRID, _C_GRID = _build_fit()


def _coef_eval(w):
    """Evaluate coefficient functions (rows 1..R+1: linear + R basis) at w."""
    wc = np.clip(w, _W_GRID[0], _W_GRID[-1])
    out = np.empty((R + 1,) + w.shape, dtype=np.float64)
    for i in range(R + 1):
        out[i] = np.interp(wc, _W_GRID, _C_GRID[i + 1])
    return out


def _ceil(x, m):
    return -(-x // m) * m


# ---------------------------------------------------------------------------
# device kernel
# ---------------------------------------------------------------------------


def build_kernel(layout, bound):
    """layout: tuple of segments, each a tuple of slot widths (le_i, padded so
    each segment total is a multiple of 64 -- pad carried by the last slot's
    mask only; slot widths themselves are the DMA/apply sizes)."""
    segs = [list(s) for s in layout]
    seg_T = [sum(s) for s in segs]
    T = sum(seg_T)
    seg_off = np.concatenate([[0], np.cumsum(seg_T)]).astype(int)
    assert all(t % 64 == 0 and t <= SEG_CAP for t in seg_T)
    nseg = len(segs)

    # global slot offsets/widths (in token axis), slot index = (seg, j)
    slot_off = []
    slot_w = []
    for si, s in enumerate(segs):
        o = int(seg_off[si])
        for wdt in s:
            slot_off.append(o)
            slot_w.append(int(wdt))
            o += int(wdt)
    nslots = len(slot_off)
    assert nslots == Q

    nc = bacc.Bacc("TRN2", target_bir_lowering=False, debug=False)

    x_dram = nc.dram_tensor("x", [T, D], BF16, kind="ExternalInput")
    m_dram = nc.dram_tensor("m_row_in", [1, T], BF16, kind="ExternalInput")
    uwt_dram = nc.dram_tensor("uwT", [128, DC * A], BF16, kind="ExternalInput")
    co_dram = nc.dram_tensor("coefs", [A, (R + 1) * P], BF16, kind="ExternalInput")
    out_dram = nc.dram_tensor("out", [Q, P, D], BF16, kind="ExternalOutput")
    den_dram = nc.dram_tensor("den", [P, nseg], F32, kind="ExternalOutput")
    if DEBUG_DUMP:
        uh_dbg = nc.dram_tensor("uh_dbg", [A, T], BF16, kind="ExternalOutput")
        ef_dbg = nc.dram_tensor("ef_dbg", [P, T], BF16, kind="ExternalOutput")

    def chunks(lo, hi, step=512):
        return [(o, min(step, hi - o)) for o in range(lo, hi, step)]

    with TileContext(nc) as tc:
        with tc.tile_pool(name="live", bufs=1) as L:
            ident = L.tile([128, 128], F32)
            ident_b = L.tile([P, P], BF16)
            x_all = L.tile([128, Q * D], BF16)
            xT = L.tile([128, DC * T], BF16)
            uhT = L.tile([A, T], BF16)
            uwt_sb = L.tile([128, DC * A], BF16)
            co_sb = L.tile([A, (R + 1) * P], BF16)
            m_row = L.tile([1, T], BF16)
            ones_b = L.tile([1, P], BF16)
            e_full = L.tile([P, T], BF16)
            aT_all = L.tile([128, Q * P], BF16)
            sumh = L.tile([P, nseg], F32)
            bvals = L.tile([A, R], F32)
            nbnd = L.tile([P, 1], F32)

            make_identity(nc, ident)
            nc.vector.tensor_copy(ident_b[:], ident[0:P, 0:P])
            nc.gpsimd.memset(ones_b[:], 1.0)
            for r in range(R):
                nc.gpsimd.memset(bvals[:, r:r + 1], float(_B_R[r]))
            nc.gpsimd.memset(nbnd[:], -float(bound))

            # ---- input DMAs (xT first: uh is the critical path) ---------
            nc.sync.dma_start(uwt_sb[:], uwt_dram[:])
            if 'co' not in KSKIP:
                nc.sync.dma_start(co_sb[:], co_dram[:])
            if 'm' not in KSKIP:
                nc.sync.dma_start(m_row[:], m_dram[:])
            for si in range(nseg):
                for off, w in chunks(int(seg_off[si]), int(seg_off[si + 1])):
                    for rb0, rbw in chunks(off, off + w, 256):
                        for c in range(DC):
                            nc.sync.dma_start(
                                xT[:, c * T + rb0:c * T + rb0 + rbw],
                                x_dram.ap()[rb0:rb0 + rbw, c * 128:(c + 1) * 128],
                                transpose=True)
            if 'xall' not in KSKIP:
                for i in range(nslots):
                    nc.sync.dma_start(
                        x_all[0:slot_w[i], i * D:(i + 1) * D],
                        x_dram.ap()[slot_off[i]:slot_off[i] + slot_w[i], :])

            with (
                tc.tile_pool(name="scp", bufs=3) as SC,
                tc.tile_pool(name="osp", bufs=3) as OSB,
                tc.tile_pool(name="ups", bufs=1, space="PSUM") as UPS,
                tc.tile_pool(name="pse", bufs=1, space="PSUM") as PE_,
                tc.tile_pool(name="pst", bufs=2, space="PSUM") as PT,
                tc.tile_pool(name="pso", bufs=2, space="PSUM") as PO,
            ):
                # ---- uh for all tokens ----------------------------------
                for off, w in chunks(0, T):
                    ups = UPS.tile([A, 512], F32, tag="ups")
                    for c in range(DC):
                        nc.tensor.matmul(
                            ups[:, 0:w], uwt_sb[:, c * A:(c + 1) * A],
                            xT[:, c * T + off:c * T + off + w],
                            start=(c == 0), stop=(c == DC - 1))
                    nc.vector.tensor_copy(uhT[:, off:off + w], ups[:, 0:w])

                # ---- per segment: R tanh passes + e matmuls + exp -------
                slot0 = 0
                for si in range(nseg if STAGE >= 1 else 0):
                    s0, s1 = int(seg_off[si]), int(seg_off[si + 1])
                    Th = s1 - s0
                    e_ps = PE_.tile([P, SEG_CAP], F32, tag="eps")
                    for r in range(R):
                        sc = SC.tile([A, SEG_CAP], BF16, tag="sc")
                        nc.scalar.activation(
                            sc[:, 0:Th], uhT[:, s0:s1], AF.Tanh,
                            bias=bvals[:, r:r + 1], scale=1.0)
                        for off, w in (chunks(0, Th) if STAGE >= 2 else []):
                            nc.tensor.matmul(
                                e_ps[:, off:off + w],
                                co_sb[:, (r + 1) * P:(r + 2) * P],
                                sc[:, off:off + w],
                                start=(r == 0), stop=False)
                    for off, w in (chunks(0, Th) if STAGE >= 2 else []):
                        nc.tensor.matmul(
                            e_ps[:, off:off + w], co_sb[:, 0:P],
                            uhT[:, s0 + off:s0 + off + w],
                            start=False, stop=False)
                        nc.tensor.matmul(
                            e_ps[:, off:off + w], ones_b[:, 0:P],
                            m_row[:, s0 + off:s0 + off + w],
                            start=False, stop=True)
                    if STAGE >= 2:
                        nc.scalar.activation(
                            e_full[:, s0:s1], e_ps[:, 0:Th], AF.Exp,
                            bias=nbnd[:, 0:1], scale=1.0,
                            accum_out=sumh[:, si:si + 1])

                    # ---- apply for this segment's slots (3 per bank) ----
                    seg_slots = list(range(slot0, slot0 + len(segs[si])))
                    slot0 += len(segs[si])
                    for g0 in (range(0, len(seg_slots), 3) if STAGE >= 3 else []):
                        grp = seg_slots[g0:g0 + 3]
                        ops = PO.tile([128, D], F32, tag="ops")
                        for j, i in enumerate(grp):
                            le_i = slot_w[i]
                            atp = PT.tile([128, P], BF16, tag="atp")
                            nc.tensor.transpose(
                                atp[0:le_i, :],
                                e_full[:, slot_off[i]:slot_off[i] + le_i],
                                ident_b[:])
                            nc.vector.tensor_copy(
                                aT_all[0:le_i, i * P:(i + 1) * P], atp[0:le_i, :])
                            nc.tensor.matmul(
                                ops[32 * j:32 * (j + 1), :],
                                aT_all[0:le_i, i * P:(i + 1) * P],
                                x_all[0:le_i, i * D:(i + 1) * D],
                                start=True, stop=True)
                        osb = OSB.tile([128, D], BF16, tag="osb")
                        nc.vector.tensor_copy(osb[0:32 * len(grp), :],
                                              ops[0:32 * len(grp), :])
                        nc.sync.dma_start(
                            out_dram.ap().rearrange("s p d -> (s p) d")[
                                grp[0] * P:(grp[-1] + 1) * P, :],
                            osb[0:32 * len(grp), :])

            if DEBUG_DUMP:
                nc.sync.dma_start(uh_dbg[:], uhT[:])
                if STAGE >= 2:
                    nc.sync.dma_start(ef_dbg[:], e_full[:])
            if STAGE >= 2:
                nc.sync.dma_start(den_dram[:], sumh[:])
            else:
                nc.gpsimd.memset(sumh[:], 1.0)
                nc.sync.dma_start(den_dram[:], sumh[:])

    nc.compile()
    return nc


_NC_CACHE = {}
LAST_NC = None


def _get_nc(layout, bound):
    key = (layout, round(float(bound), 3))
    if key not in _NC_CACHE:
        _NC_CACHE[key] = build_kernel(layout, bound)
    return _NC_CACHE[key]


# ---------------------------------------------------------------------------
# host entry point
# ---------------------------------------------------------------------------


def _make_layout(le_prof):
    """Split the descending per-rank profile into segments with padded total
    <= SEG_CAP each, balancing totals; widths padded so each segment total is
    a multiple of 64 (pad added to the last slot of the segment)."""
    tot = int(le_prof.sum())
    nseg = max(2, int(np.ceil(_ceil(tot, 64) / SEG_CAP)))
    # balanced greedy split points on the prefix sums
    pref = np.concatenate([[0], np.cumsum(le_prof)])
    bounds = [0]
    for k in range(1, nseg):
        target = tot * k / nseg
        bounds.append(int(np.argmin(np.abs(pref - target))))
    bounds.append(Q)
    segs = []
    for k in range(nseg):
        wdts = [int(v) for v in le_prof[bounds[k]:bounds[k + 1]]]
        pad = int(_ceil(sum(wdts), 64) - sum(wdts))
        wdts[-1] += pad
        assert sum(wdts) <= SEG_CAP
        segs.append(tuple(wdts))
    return tuple(segs)


def kernel(exp_tokens, exp_mask, s_j, req_mask, Ws_w, Ws_b, U_w, v_w):
    """Full-input entry point: shard over B across 8 cores, gather output."""
    from concourse.bass_utils import run_bass_kernel_spmd

    exp_tokens = np.asarray(exp_tokens, dtype=np.float32)
    exp_mask = np.asarray(exp_mask, dtype=np.int32)
    s_j = np.asarray(s_j, dtype=np.float32)
    req_mask = np.asarray(req_mask, dtype=np.int32)
    Ws_w = np.asarray(Ws_w, dtype=np.float32)
    Ws_b = np.asarray(Ws_b, dtype=np.float32)
    U_w = np.asarray(U_w, dtype=np.float32)
    v_w = np.asarray(v_w, dtype=np.float32)

    # ---- per-rank compaction profile ------------------------------------
    counts = exp_mask.sum(axis=2)                      # [B, Q]
    order = np.argsort(-counts, axis=1, kind="stable")
    sorted_counts = np.take_along_axis(counts, order, axis=1)
    le_prof = sorted_counts.max(axis=0)                # [Q]
    layout = _make_layout(le_prof)
    slot_w = [w for s in layout for w in s]
    slot_off = np.concatenate([[0], np.cumsum(slot_w)]).astype(int)
    T = int(slot_off[-1])

    # ---- compacted x + mask row ----------------------------------------
    x_c = np.zeros((B, T, D), dtype=np.float32)
    m_row = np.full((B, 1, T), -1e9, dtype=np.float32)
    for b in range(B):
        for i in range(Q):
            qo = order[b, i]
            idx = np.flatnonzero(exp_mask[b, qo])
            n = len(idx)
            o = slot_off[i]
            x_c[b, o:o + n] = exp_tokens[b, qo, idx]
            m_row[b, 0, o:o + n] = 0.0

    # ---- host coefficients ---------------------------------------------
    ws = np.einsum("bpd,ad->bpa", s_j, Ws_w, optimize=True) + Ws_b  # [B,P,A]
    co = _coef_eval(ws) * v_w[0][None, None, None, :]  # [(R+1), B, P, A]
    # stationary layout [A, (R+1)*P], order: linear first, then basis r
    coefs = np.ascontiguousarray(
        np.transpose(co, (1, 3, 0, 2)).reshape(B, A, (R + 1) * P))
    bound = float(np.abs(co[1:]).sum(axis=(0, 3)).max()
                  + 6.0 * np.abs(co[0]).sum(axis=2).max()) + 1.0
    bound = _ceil(bound, 4.0)

    # uwT: [128, DC*A] with uwT[dd, c*A+a] = U_w[a, c*128+dd]
    uwT = np.ascontiguousarray(
        U_w.reshape(A, DC, 128).transpose(2, 1, 0).reshape(128, DC * A))

    nc = _get_nc(layout, bound)
    global LAST_NC
    LAST_NC = nc

    x_bf = x_c.astype(ml_dtypes.bfloat16)
    uwT_bf = uwT.astype(ml_dtypes.bfloat16)
    coefs_bf = coefs.astype(ml_dtypes.bfloat16)
    in_maps = []
    for b in range(B):
        in_maps.append({
            "x": x_bf[b],
            "m_row_in": m_row[b].astype(ml_dtypes.bfloat16),
            "uwT": uwT_bf,
            "coefs": coefs_bf[b],
        })
    res = run_bass_kernel_spmd(nc, in_maps, core_ids=list(range(N_CORES)))

    out = np.empty((B, Q, P, D), dtype=np.float32)
    for b in range(B):
        o_slot = res.results[b]["out"].astype(np.float32)   # [Q, P, D]
        den = res.results[b]["den"].astype(np.float64).sum(axis=1)  # [P]
        scale = (req_mask[b].astype(np.float64) / (den + 1e-300)).astype(np.float32)
        o_slot *= scale[None, :, None]
        out[b, order[b]] = o_slot
    return out


# revision 15
# speedup vs baseline: 2.1175x; 1.2183x over previous
"""Trainium2 Bass kernel for nn_AbilityGammaAttention.

Reference computation (per batch b):
    ws = s_j @ Ws_w.T + Ws_b                      # (P, A)
    uh = exp_tokens @ U_w.T                       # (Q, LE, A)
    e[q,p,t] = v . tanh(uh[q,t,:] + ws[p,:])      # (Q, P, LE)
    e masked by exp_mask (tokens), joint softmax over (Q, LE) per (b, p)
    out[q,p,:] = sum_t a[q,p,t] * exp_tokens[q,t,:], zeroed where req_mask[p]==0

Sharding: data-parallel over B across the 8 NeuronCores (batch b -> core b).

Key idea (replaces the per-p tanh loop of the previous version): expand the
shifted-tanh family in a fixed basis,

    tanh(u + w) ~= c0(w) + clin(w)*u + sum_r c_r(w) * tanh(u + b_r),

with R=12 fixed shifts b_r. The c*(w) coefficient functions are least-squares
fits (precomputed on a w-grid at import; Gaussian-weighted in u with a uniform
floor for tail control). Since ws = s_j@Ws_w.T + Ws_b is host-computable, the
host evaluates the coefficients at the actual w values and uploads, per core,
stationary matrices S_r[a,p] = v_a * c_r(ws[p,a]). On device:

    e[p,t] = sum_r (S_r^T @ tanh(uhT + b_r))[p,t] + (S_lin^T @ uhT)[p,t] + mask

i.e. R ScalarE tanh passes over [A, T] + (R+2) PE matmuls, instead of P=32
tanh passes. The c0 term is a per-p constant -> cancels in the softmax over t
(the denominators come from the Exp pass's accum_out and the normalization +
req_mask is applied on the host).

Other structure:
  - Host-side token compaction with a per-rank profile: each batch's queries
    are sorted by unmasked-token count; slot i is padded to the max count at
    rank i across batches (shared static shape, ~18% fewer tokens than
    uniform padding). Padding slots are masked via an additive (m-1)*1e9
    rank-1 matmul, exactly like the reference -1e9 masking.
  - All heavy dataflow in bf16: x is uploaded bf16 and XBAR-transpose-DMA'd
    into xT (no on-device transposes for uh), uh/tanh/coefficients/apply all
    bf16 (f32 PSUM accumulation).
  - Output: per-slot apply matmuls pack 3 slots per PSUM bank (base
    partitions 0/32/64), evacuated bf16 and DMA'd out unnormalized; the host
    divides by the denominators and applies req_mask.
"""

import sys

if "/opt/trn_rl_repo" not in sys.path:
    sys.path.insert(0, "/opt/trn_rl_repo")

import numpy as np
import ml_dtypes

import concourse.bacc as bacc
import concourse.mybir as mybir
from concourse.masks import make_identity
from concourse.tile import TileContext

F32 = mybir.dt.float32
F32R = mybir.dt.float32r
BF16 = mybir.dt.bfloat16
AF = mybir.ActivationFunctionType

B, Q, LE, D, P, A = 8, 32, 128, 512, 32, 128
N_CORES = 8
DC = D // 128
R = 12                       # tanh basis size
DEBUG_DUMP = False
STAGE = 3
import os as _os
KSKIP = set(_os.environ.get('KSKIP','').split(','))  # debug: 0=uh only, 1=+tanh, 2=+e/exp, 3=full
SEG_CAP = 1536               # max tokens per segment (3 PSUM banks of f32)

# ---------------------------------------------------------------------------
# basis fit (data-independent; computed once at import)
# ---------------------------------------------------------------------------


def _build_fit(r=R, ridge=1e-6):
    b_r = np.linspace(-4.4, 4.4, r)
    u = np.linspace(-6.5, 6.5, 1301)
    rho = np.exp(-0.5 * u * u) + 0.01
    rho /= rho.sum()
    Phi = np.concatenate(
        [np.ones_like(u)[:, None], u[:, None], np.tanh(u[:, None] + b_r[None, :])],
        axis=1)
    M = Phi.T @ (rho[:, None] * Phi) + ridge * np.eye(r + 2)
    w_grid = np.linspace(-5.0, 5.0, 2001)
    G = np.tanh(u[:, None] + w_grid[None, :])
    C_grid = np.linalg.solve(M, Phi.T @ (rho[:, None] * G))   # [(r+2), Nw]
    return b_r, w_grid, C_grid


_B_R, _W_GRID, _C_GRID = _build_fit()


def _coef_eval(w):
    """Evaluate coefficient functions (rows 1..R+1: linear + R basis) at w."""
    wc = np.clip(w, _W_GRID[0], _W_GRID[-1])
    out = np.empty((R + 1,) + w.shape, dtype=np.float64)
    for i in range(R + 1):
        out[i] = np.interp(wc, _W_GRID, _C_GRID[i + 1])
    return out


def _ceil(x, m):
    return -(-x // m) * m


# ---------------------------------------------------------------------------
# device kernel
# ---------------------------------------------------------------------------


def build_kernel(layout, bound):
    """layout: tuple of segments, each a tuple of slot widths (le_i, padded so
    each segment total is a multiple of 64 -- pad carried by the last slot's
    mask only; slot widths themselves are the DMA/apply sizes)."""
    segs = [list(s) for s in layout]
    seg_T = [sum(s) for s in segs]
    T = sum(seg_T)
    seg_off = np.concatenate([[0], np.cumsum(seg_T)]).astype(int)
    assert all(t % 64 == 0 and t <= SEG_CAP for t in seg_T)
    nseg = len(segs)

    # global slot offsets/widths (in token axis), slot index = (seg, j)
    slot_off = []
    slot_w = []
    for si, s in enumerate(segs):
        o = int(seg_off[si])
        for wdt in s:
            slot_off.append(o)
            slot_w.append(int(wdt))
            o += int(wdt)
    nslots = len(slot_off)
    assert nslots == Q

    nc = bacc.Bacc("TRN2", target_bir_lowering=False, debug=False)

    x_dram = nc.dram_tensor("x", [T, D], BF16, kind="ExternalInput")
    xp_dram = nc.dram_tensor("x_pad", [128, Q * D], BF16, kind="ExternalInput")
    m_dram = nc.dram_tensor("m_row_in", [1, T], BF16, kind="ExternalInput")
    uwt_dram = nc.dram_tensor("uwT", [128, DC * A], BF16, kind="ExternalInput")
    co_dram = nc.dram_tensor("coefs", [A, (R + 1) * P], BF16, kind="ExternalInput")
    out_dram = nc.dram_tensor("out", [Q, P, D], BF16, kind="ExternalOutput")
    den_dram = nc.dram_tensor("den", [P, nseg], F32, kind="ExternalOutput")
    if DEBUG_DUMP:
        uh_dbg = nc.dram_tensor("uh_dbg", [A, T], BF16, kind="ExternalOutput")
        ef_dbg = nc.dram_tensor("ef_dbg", [P, T], BF16, kind="ExternalOutput")

    def chunks(lo, hi, step=512):
        return [(o, min(step, hi - o)) for o in range(lo, hi, step)]

    with TileContext(nc) as tc:
        with tc.tile_pool(name="live", bufs=1) as L:
            ident = L.tile([128, 128], F32)
            ident_b = L.tile([P, P], BF16)
            x_all = L.tile([128, Q * D], BF16)
            xT = L.tile([128, DC * T], BF16)
            uhT = L.tile([A, T], BF16)
            uwt_sb = L.tile([128, DC * A], BF16)
            co_sb = L.tile([A, (R + 1) * P], BF16)
            m_row = L.tile([1, T], BF16)
            ones_b = L.tile([1, P], BF16)
            e_full = L.tile([P, T], BF16)
            aT_all = L.tile([128, Q * P], BF16)
            sumh = L.tile([P, nseg], F32)
            bvals = L.tile([A, R], F32)
            nbnd = L.tile([P, 1], F32)

            make_identity(nc, ident)
            nc.vector.tensor_copy(ident_b[:], ident[0:P, 0:P])
            nc.gpsimd.memset(ones_b[:], 1.0)
            for r in range(R):
                nc.gpsimd.memset(bvals[:, r:r + 1], float(_B_R[r]))
            nc.gpsimd.memset(nbnd[:], -float(bound))

            # ---- input DMAs (xT first: uh is the critical path) ---------
            nc.sync.dma_start(uwt_sb[:], uwt_dram[:])
            if 'co' not in KSKIP:
                nc.sync.dma_start(co_sb[:], co_dram[:])
            if 'm' not in KSKIP:
                nc.sync.dma_start(m_row[:], m_dram[:])
            xblks = chunks(0, T, _ceil(T // 2, 16))
            for rb0, rbw in xblks:
                for c in range(DC):
                    nc.sync.dma_start(
                        xT[:, c * T + rb0:c * T + rb0 + rbw],
                        x_dram.ap()[rb0:rb0 + rbw, c * 128:(c + 1) * 128],
                        transpose=True)
            if 'xall' not in KSKIP:
                nc.sync.dma_start(x_all[:], xp_dram[:])

            with (
                tc.tile_pool(name="scp", bufs=3) as SC,
                tc.tile_pool(name="osp", bufs=3) as OSB,
                tc.tile_pool(name="ups", bufs=1, space="PSUM") as UPS,
                tc.tile_pool(name="pse", bufs=1, space="PSUM") as PE_,
                tc.tile_pool(name="pst", bufs=2, space="PSUM") as PT,
                tc.tile_pool(name="pso", bufs=2, space="PSUM") as PO,
            ):
                # ---- uh for all tokens ----------------------------------
                for off, w in chunks(0, T):
                    ups = UPS.tile([A, 512], F32, tag="ups")
                    for c in range(DC):
                        nc.tensor.matmul(
                            ups[:, 0:w], uwt_sb[:, c * A:(c + 1) * A],
                            xT[:, c * T + off:c * T + off + w],
                            start=(c == 0), stop=(c == DC - 1))
                    nc.vector.tensor_copy(uhT[:, off:off + w], ups[:, 0:w])

                # ---- per segment: R tanh passes + e matmuls + exp -------
                slot0 = 0
                for si in range(nseg if STAGE >= 1 else 0):
                    s0, s1 = int(seg_off[si]), int(seg_off[si + 1])
                    Th = s1 - s0
                    e_ps = PE_.tile([P, SEG_CAP], F32, tag="eps")
                    for r in range(R):
                        sc = SC.tile([A, SEG_CAP], BF16, tag="sc")
                        nc.scalar.activation(
                            sc[:, 0:Th], uhT[:, s0:s1], AF.Tanh,
                            bias=bvals[:, r:r + 1], scale=1.0)
                        for off, w in (chunks(0, Th) if STAGE >= 2 else []):
                            nc.tensor.matmul(
                                e_ps[:, off:off + w],
                                co_sb[:, (r + 1) * P:(r + 2) * P],
                                sc[:, off:off + w],
                                start=(r == 0), stop=False)
                    for off, w in (chunks(0, Th) if STAGE >= 2 else []):
                        nc.tensor.matmul(
                            e_ps[:, off:off + w], co_sb[:, 0:P],
                            uhT[:, s0 + off:s0 + off + w],
                            start=False, stop=False)
                        nc.tensor.matmul(
                            e_ps[:, off:off + w], ones_b[:, 0:P],
                            m_row[:, s0 + off:s0 + off + w],
                            start=False, stop=True)
                    if STAGE >= 2:
                        nc.scalar.activation(
                            e_full[:, s0:s1], e_ps[:, 0:Th], AF.Exp,
                            bias=nbnd[:, 0:1], scale=1.0,
                            accum_out=sumh[:, si:si + 1])

                    # ---- apply for this segment's slots (3 per bank) ----
                    seg_slots = list(range(slot0, slot0 + len(segs[si])))
                    slot0 += len(segs[si])
                    groups = [seg_slots[g0:g0 + 3]
                              for g0 in range(0, len(seg_slots), 3)]
                    if STAGE < 3:
                        groups = []

                    def do_group(grp, ops):
                        for j, i in enumerate(grp):
                            le_i = slot_w[i]
                            atp = PT.tile([128, P], BF16, tag="atp")
                            nc.tensor.transpose(
                                atp[0:le_i, :],
                                e_full[:, slot_off[i]:slot_off[i] + le_i],
                                ident_b[:])
                            nc.vector.tensor_copy(
                                aT_all[0:le_i, i * P:(i + 1) * P], atp[0:le_i, :])
                            nc.tensor.matmul(
                                ops[32 * j:32 * (j + 1), :],
                                aT_all[0:le_i, i * P:(i + 1) * P],
                                x_all[0:le_i, i * D:(i + 1) * D],
                                start=True, stop=True)

                    full = [g for g in groups if len(g) == 3]
                    rem = [g for g in groups if len(g) < 3]
                    if full:
                        nb = len(full)
                        osb = OSB.tile([128, 5 * D], BF16, tag="osb", bufs=2)
                        for j, grp in enumerate(full):
                            ops = PO.tile([128, D], F32, tag="ops")
                            do_group(grp, ops)
                            nc.vector.tensor_copy(osb[0:96, j * D:(j + 1) * D],
                                                  ops[0:96, :])
                        s0 = full[0][0]
                        nc.sync.dma_start(
                            out_dram.ap().rearrange("s p d -> (s p) d")[
                                s0 * P:s0 * P + nb * 96, :].rearrange(
                                    "(g r) d -> r g d", r=96),
                            osb[0:96, 0:nb * D].rearrange("r (g d) -> r g d", d=D))
                    for grp in rem:
                        ops = PO.tile([128, D], F32, tag="ops")
                        do_group(grp, ops)
                        osb2 = OSB.tile([128, D], BF16, tag="osb2", bufs=2)
                        nc.vector.tensor_copy(osb2[0:32 * len(grp), :],
                                              ops[0:32 * len(grp), :])
                        nc.sync.dma_start(
                            out_dram.ap().rearrange("s p d -> (s p) d")[
                                grp[0] * P:(grp[-1] + 1) * P, :],
                            osb2[0:32 * len(grp), :])

            if DEBUG_DUMP:
                nc.sync.dma_start(uh_dbg[:], uhT[:])
                if STAGE >= 2:
                    nc.sync.dma_start(ef_dbg[:], e_full[:])
            if STAGE >= 2:
                nc.sync.dma_start(den_dram[:], sumh[:])
            else:
                nc.gpsimd.memset(sumh[:], 1.0)
                nc.sync.dma_start(den_dram[:], sumh[:])

    nc.compile()
    return nc


_NC_CACHE = {}
LAST_NC = None


def _get_nc(layout, bound):
    key = (layout, round(float(bound), 3))
    if key not in _NC_CACHE:
        _NC_CACHE[key] = build_kernel(layout, bound)
    return _NC_CACHE[key]


# ---------------------------------------------------------------------------
# host entry point
# ---------------------------------------------------------------------------


def _make_layout(le_prof):
    """Split the descending per-rank profile into segments with padded total
    <= SEG_CAP each, balancing totals; widths padded so each segment total is
    a multiple of 64 (pad added to the last slot of the segment)."""
    tot = int(le_prof.sum())
    nseg = max(2, int(np.ceil(_ceil(tot, 64) / SEG_CAP)))
    # balanced greedy split points on the prefix sums
    pref = np.concatenate([[0], np.cumsum(le_prof)])
    bounds = [0]
    for k in range(1, nseg):
        target = tot * k / nseg
        bounds.append(int(np.argmin(np.abs(pref - target))))
    bounds.append(Q)
    segs = []
    for k in range(nseg):
        wdts = [int(v) for v in le_prof[bounds[k]:bounds[k + 1]]]
        pad = int(_ceil(sum(wdts), 64) - sum(wdts))
        wdts[-1] += pad
        assert sum(wdts) <= SEG_CAP
        segs.append(tuple(wdts))
    return tuple(segs)


def kernel(exp_tokens, exp_mask, s_j, req_mask, Ws_w, Ws_b, U_w, v_w):
    """Full-input entry point: shard over B across 8 cores, gather output."""
    from concourse.bass_utils import run_bass_kernel_spmd

    exp_tokens = np.asarray(exp_tokens, dtype=np.float32)
    exp_mask = np.asarray(exp_mask, dtype=np.int32)
    s_j = np.asarray(s_j, dtype=np.float32)
    req_mask = np.asarray(req_mask, dtype=np.int32)
    Ws_w = np.asarray(Ws_w, dtype=np.float32)
    Ws_b = np.asarray(Ws_b, dtype=np.float32)
    U_w = np.asarray(U_w, dtype=np.float32)
    v_w = np.asarray(v_w, dtype=np.float32)

    # ---- per-rank compaction profile ------------------------------------
    counts = exp_mask.sum(axis=2)                      # [B, Q]
    order = np.argsort(-counts, axis=1, kind="stable")
    sorted_counts = np.take_along_axis(counts, order, axis=1)
    le_prof = sorted_counts.max(axis=0)                # [Q]
    layout = _make_layout(le_prof)
    slot_w = [w for s in layout for w in s]
    slot_off = np.concatenate([[0], np.cumsum(slot_w)]).astype(int)
    T = int(slot_off[-1])

    # ---- compacted x + mask row ----------------------------------------
    x_c = np.zeros((B, T, D), dtype=np.float32)
    m_row = np.full((B, 1, T), -1e9, dtype=np.float32)
    for b in range(B):
        for i in range(Q):
            qo = order[b, i]
            idx = np.flatnonzero(exp_mask[b, qo])
            n = len(idx)
            o = slot_off[i]
            x_c[b, o:o + n] = exp_tokens[b, qo, idx]
            m_row[b, 0, o:o + n] = 0.0

    # ---- host coefficients ---------------------------------------------
    ws = np.einsum("bpd,ad->bpa", s_j, Ws_w, optimize=True) + Ws_b  # [B,P,A]
    co = _coef_eval(ws) * v_w[0][None, None, None, :]  # [(R+1), B, P, A]
    # stationary layout [A, (R+1)*P], order: linear first, then basis r
    coefs = np.ascontiguousarray(
        np.transpose(co, (1, 3, 0, 2)).reshape(B, A, (R + 1) * P))
    bound = float(np.abs(co[1:]).sum(axis=(0, 3)).max()
                  + 6.0 * np.abs(co[0]).sum(axis=2).max()) + 1.0
    bound = _ceil(bound, 4.0)

    # uwT: [128, DC*A] with uwT[dd, c*A+a] = U_w[a, c*128+dd]
    uwT = np.ascontiguousarray(
        U_w.reshape(A, DC, 128).transpose(2, 1, 0).reshape(128, DC * A))

    nc = _get_nc(layout, bound)
    global LAST_NC
    LAST_NC = nc

    x_bf = x_c.astype(ml_dtypes.bfloat16)
    x_pad = np.zeros((B, 128, Q, D), dtype=np.float32)
    for i in range(Q):
        o, wdt = int(slot_off[i]), slot_w[i]
        x_pad[:, 0:wdt, i, :] = x_c[:, o:o + wdt, :]
    x_pad_bf = x_pad.reshape(B, 128, Q * D).astype(ml_dtypes.bfloat16)
    uwT_bf = uwT.astype(ml_dtypes.bfloat16)
    coefs_bf = coefs.astype(ml_dtypes.bfloat16)
    in_maps = []
    for b in range(B):
        in_maps.append({
            "x": x_bf[b],
            "x_pad": x_pad_bf[b],
            "m_row_in": m_row[b].astype(ml_dtypes.bfloat16),
            "uwT": uwT_bf,
            "coefs": coefs_bf[b],
        })
    res = run_bass_kernel_spmd(nc, in_maps, core_ids=list(range(N_CORES)))

    out = np.empty((B, Q, P, D), dtype=np.float32)
    for b in range(B):
        o_slot = res.results[b]["out"].astype(np.float32)   # [Q, P, D]
        den = res.results[b]["den"].astype(np.float64).sum(axis=1)  # [P]
        scale = (req_mask[b].astype(np.float64) / (den + 1e-300)).astype(np.float32)
        o_slot *= scale[None, :, None]
        out[b, order[b]] = o_slot
    return out
